# revision 4
# baseline (speedup 1.0000x reference)
"""Trainium2 kernel for nn_PatternsOfThinkingBlock (topk_masking), v2.

reference:
  idx = argmax(x, -1); gathered = x[..., idx]   (gathered == row max)
  y = gelu(einsum('bhs,ts->bht', gathered, W) + b)   (exact erf gelu)
  out = x with x[b,h,s,idx[b,h,s]] = y[b,h,s]

Strategy vs the v1 baseline: the output tolerance (2e-2 of absmax) lets the
output travel as fp16 (roundoff ~4e-3 absolute), halving write traffic; the
max/argmax stay exact in f32.  Pure data parallel over the 32 (b,h) slices,
4 per core.  Per [2048, 2048] slice, per 128-row chunk of the stream:
  - HWDGE DMA-in (f32, 1 MB per chunk)
  - DVE max8 -> exact f32 row max; DVE max_index -> exact FIRST-occurrence
    argmax (the dataset has 4 rows with exactly tied f32 maxima; first
    occurrence must match the reference argmax)
  - ACT copies the chunk to fp16 (casting off the busy DVE)
Slice level: PE matvec z = W @ rowmax against resident fp16 W.T chunks
(fed per chunk as each max lands), gelu+bias on ACT/DVE, then per chunk a
4x-mode tensor_scalar builds onehot(idx)*(y - m) in fp16 from a resident
iota and a 2x tensor_tensor adds it into the fp16 chunk in place; HWDGE
DMA-out (fp16, 1 MB per 2-chunk block).  DMA moves 16.8 MB in + 8.4 MB out
per slice (~71 us serial), under the DVE scan floor of ~98 us/slice --
the two first-occurrence-exact scans are the binding constraint on TRN2's
real ISA (no elementwise ops on Pool, no fused max+index op).
"""

import numpy as np

import concourse.bacc as bacc
import concourse.mybir as mybir
import concourse.tile as tile
from concourse import bass_utils

F32 = mybir.dt.float32
F16 = mybir.dt.float16
I16 = mybir.dt.int16
U32 = mybir.dt.uint32

AF = mybir.ActivationFunctionType
ALU = mybir.AluOpType

S = 2048
NSL = 4            # bh slices per core
N_CORES = 8
C = S // 128       # 16 row chunks per slice


def _build(n_cores=N_CORES, repeat=1, internal_io=False, nsl=NSL,
           xbufs=6, hbufs=C // 2 + 1, dma_apply=0, tbufs=2):
    nc = bacc.Bacc("TRN2", target_bir_lowering=False, debug=False,
                   num_devices=n_cores)

    big_in = "Internal" if internal_io else "ExternalInput"
    big_out = "Internal" if internal_io else "ExternalOutput"
    xs = nc.dram_tensor("xs", (nsl, S, S), F32, kind=big_in).ap()
    wt = nc.dram_tensor("wt", (S, S), F16, kind=big_in).ap()
    bias = nc.dram_tensor("bias", (S,), F32, kind=big_in).ap()
    outs = [nc.dram_tensor(f"out{n}", (S, S), F16, kind=big_out).ap()
            for n in range(nsl)]
    if internal_io:
        dum_in = nc.dram_tensor("dum_in", (128, 4), F32,
                                kind="ExternalInput").ap()
        dum_out = nc.dram_tensor("dum_out", (128, nsl + 4), F16,
                                 kind="ExternalOutput").ap()

    with tile.TileContext(nc) as tc:
        with tc.tile_pool(name="res", bufs=1) as rpool, \
             tc.tile_pool(name="xf", bufs=xbufs) as xpool, \
             tc.tile_pool(name="xh", bufs=hbufs) as hpool, \
             tc.tile_pool(name="t16", bufs=tbufs) as tpool, \
             tc.tile_pool(name="sl", bufs=2) as slpool, \
             tc.tile_pool(name="psum", bufs=2, space="PSUM") as ppool, \
             tc.tile_pool(name="ionce", bufs=1) as ipool, \
             tc.tile_pool(name="small", bufs=2) as spool:

            wt_sb = rpool.tile([128, C * S], F16)
            bias_sb = rpool.tile([128, C], F32)
            iota16 = rpool.tile([128, S], F16)   # 0..2047 per partition

            ii = ipool.tile([128, S], I16, tag="iota_i")
            nc.gpsimd.iota(ii[:], pattern=[[1, S]], base=0,
                           channel_multiplier=0)
            nc.vector.tensor_copy(iota16[:], ii[:])
            # weight/bias loads ride the ACT HWDGE queue so the first slice's
            # input stream (SP queue) interleaves with them on the DMA device
            for c in range(C):
                nc.scalar.dma_start(wt_sb[:, c * S:(c + 1) * S],
                                    wt[c * 128:(c + 1) * 128, :])
            nc.scalar.dma_start(bias_sb[:],
                                bias.rearrange("(c p) -> p c", p=128))

            for it in range(nsl * repeat):
                n = it % nsl
                xn = xs[n]
                on = outs[n]

                m8 = slpool.tile([128, C * 8], F32, tag="m8")
                i8 = slpool.tile([128, C * 8], U32, tag="i8")
                gh = spool.tile([128, C], F16, tag="gh")
                psum_t = ppool.tile([128, C * C], F32, tag="psum_y")
                x16blks = []
                for c in range(C):
                    xt = xpool.tile([128, S], F32, tag="x")
                    nc.sync.dma_start(xt[:], xn[c * 128:(c + 1) * 128, :])
                    # row max then exact first-occurrence index (both DVE:
                    # the only engine the real ISA gives for scans)
                    nc.vector.max(m8[:, c * 8:(c + 1) * 8], xt[:])
                    nc.vector.max_index(i8[:, c * 8:(c + 1) * 8],
                                        m8[:, c * 8:(c + 1) * 8], xt[:])
                    if c % 2 == 0:
                        x16 = hpool.tile([128, 2 * S], F16, tag="x16")
                        x16blks.append(x16)
                    nc.scalar.activation(x16[:, (c % 2) * S:(c % 2 + 1) * S],
                                         xt[:], AF.Copy)
                    # feed PE as soon as this chunk's row max is known
                    # (fp16 cast of the max rides ACT, not the busy DVE)
                    nc.scalar.activation(gh[:, c:c + 1],
                                         m8[:, c * 8:c * 8 + 1], AF.Copy)
                    for tci in range(C):
                        nc.tensor.matmul(
                            psum_t[:, c * C + tci: c * C + tci + 1],
                            wt_sb[:, c * S + tci * 128:
                                  c * S + (tci + 1) * 128],
                            gh[:, c:c + 1],
                            start=True, stop=True)

                # col-0 strided views [128, C, 1] of the per-chunk top-8 tiles
                m3 = m8[:].rearrange("p (c e) -> p c e", e=8)[:, :, 0:1]
                i3 = i8[:].rearrange("p (c e) -> p c e", e=8)[:, :, 0:1]

                y_pre = spool.tile([128, C], F32, tag="ypre")
                nc.vector.reduce_sum(
                    y_pre[:].rearrange("p (t o) -> p t o", o=1),
                    psum_t[:].rearrange("p (s t) -> p t s", s=C),
                    axis=mybir.AxisListType.X)
                nc.vector.tensor_add(y_pre[:], y_pre[:], bias_sb[:])
                yv = spool.tile([128, C], F32, tag="yv")
                nc.scalar.activation(yv[:], y_pre[:], AF.Gelu)

                dltf = spool.tile([128, C], F32, tag="dlt")
                nc.vector.tensor_sub(
                    dltf[:].rearrange("p (c o) -> p c o", o=1),
                    yv[:].rearrange("p (c o) -> p c o", o=1), m3)
                idxf = spool.tile([128, C], F32, tag="idx")
                nc.vector.tensor_copy(
                    idxf[:].rearrange("p (c o) -> p c o", o=1), i3)

                for c in range(C):
                    t16 = tpool.tile([128, S], F16, tag="t16")
                    nc.vector.tensor_scalar(
                        t16[:], iota16[:], idxf[:, c:c + 1],
                        dltf[:, c:c + 1],
                        op0=ALU.is_equal, op1=ALU.mult)
                    blk = x16blks[c // 2]
                    half = (c % 2) * S
                    if c < dma_apply:
                        # DVE is the bottleneck: ride the DMA slack instead —
                        # SWDGE CCE accumulate-add folds t16 into the block
                        nc.gpsimd.dma_start(blk[:, half:half + S], t16[:],
                                            accum_op=ALU.add)
                    else:
                        nc.vector.tensor_add(blk[:, half:half + S],
                                             blk[:, half:half + S], t16[:])
                    if c % 2 == 1:
                        b = c // 2
                        nc.scalar.dma_start(
                            on[b * 256:(b + 1) * 256, :].rearrange(
                                "(a p) m -> p a m", p=128),
                            blk[:].rearrange("p (a m) -> p a m", a=2))

            if internal_io:
                live = spool.tile([128, nsl + 4], F16, tag="live")
                nc.gpsimd.dma_start(live[:, nsl:], dum_in[:])
                for n in range(nsl):
                    nc.sync.dma_start(live[:, n:n + 1], outs[n][:128, 0:1])
                nc.sync.dma_start(dum_out[:], live[:])

    nc.compile()
    return nc


_NC_CACHE = {}


def _get_nc():
    if "nc" not in _NC_CACHE:
        _NC_CACHE["nc"] = _build()
    return _NC_CACHE["nc"]


def _make_in_maps(x, W, b):
    x = np.ascontiguousarray(np.asarray(x, dtype=np.float32))
    W = np.asarray(W, dtype=np.float32)
    b = np.ascontiguousarray(np.asarray(b, dtype=np.float32))
    wt = np.ascontiguousarray(W.T.astype(np.float16))

    xf = x.reshape(-1, S, S)
    assert xf.shape[0] == N_CORES * NSL
    in_maps = []
    for core in range(N_CORES):
        in_maps.append({
            "xs": xf[core * NSL:(core + 1) * NSL],
            "wt": wt,
            "bias": b,
        })
    return in_maps


def _run(in_maps, **kwargs):
    nc = _get_nc()
    return bass_utils.run_bass_kernel_spmd(
        nc, in_maps, core_ids=list(range(N_CORES)), **kwargs)


def kernel(x, W, b):
    shape = np.asarray(x).shape
    res = _run(_make_in_maps(x, W, b))
    parts = [res.results[core][f"out{n}"]
             for core in range(N_CORES) for n in range(NSL)]
    return np.stack(parts).astype(np.float32).reshape(shape)


# revision 5
# speedup vs baseline: 1.0552x; 1.0552x over previous
"""Trainium2 kernel for nn_PatternsOfThinkingBlock (topk_masking), v2.

reference:
  idx = argmax(x, -1); gathered = x[..., idx]   (gathered == row max)
  y = gelu(einsum('bhs,ts->bht', gathered, W) + b)   (exact erf gelu)
  out = x with x[b,h,s,idx[b,h,s]] = y[b,h,s]

Strategy vs the v1 baseline: the output tolerance (2e-2 of absmax) lets the
output travel as fp16 (roundoff ~4e-3 absolute), halving write traffic; the
max/argmax stay exact in f32.  Pure data parallel over the 32 (b,h) slices,
4 per core.  Per [2048, 2048] slice, per 128-row chunk of the stream:
  - HWDGE DMA-in (f32, 1 MB per chunk)
  - DVE max8 -> exact f32 row max; DVE max_index -> exact FIRST-occurrence
    argmax (the dataset has 4 rows with exactly tied f32 maxima; first
    occurrence must match the reference argmax)
  - ACT copies the chunk to fp16 (casting off the busy DVE)
Slice level: PE matvec z = W @ rowmax against resident fp16 W.T chunks
(fed per chunk as each max lands), gelu+bias on ACT/DVE, then per chunk a
4x-mode tensor_scalar builds onehot(idx)*(y - m) in fp16 from a resident
iota and a 2x tensor_tensor adds it into the fp16 chunk in place; HWDGE
DMA-out (fp16, 1 MB per 2-chunk block).  DMA moves 16.8 MB in + 8.4 MB out
per slice (~71 us serial), under the DVE scan floor of ~98 us/slice --
the two first-occurrence-exact scans are the binding constraint on TRN2's
real ISA (no elementwise ops on Pool, no fused max+index op).
"""

import numpy as np

import concourse.bacc as bacc
import concourse.mybir as mybir
import concourse.tile as tile
from concourse import bass_utils

F32 = mybir.dt.float32
F16 = mybir.dt.float16
I16 = mybir.dt.int16
U32 = mybir.dt.uint32

AF = mybir.ActivationFunctionType
ALU = mybir.AluOpType

S = 2048
NSL = 4            # bh slices per core
N_CORES = 8
C = S // 128       # 16 row chunks per slice


def _build(n_cores=N_CORES, repeat=1, internal_io=False, nsl=NSL,
           xbufs=5, hbufs=C // 2 + 1, dma_apply=-5, tbufs=4):
    nc = bacc.Bacc("TRN2", target_bir_lowering=False, debug=False,
                   num_devices=n_cores)

    big_in = "Internal" if internal_io else "ExternalInput"
    big_out = "Internal" if internal_io else "ExternalOutput"
    xs = nc.dram_tensor("xs", (nsl, S, S), F32, kind=big_in).ap()
    wt = nc.dram_tensor("wt", (S, S), F16, kind=big_in).ap()
    bias = nc.dram_tensor("bias", (S,), F32, kind=big_in).ap()
    outs = [nc.dram_tensor(f"out{n}", (S, S), F16, kind=big_out).ap()
            for n in range(nsl)]
    if internal_io:
        dum_in = nc.dram_tensor("dum_in", (128, 4), F32,
                                kind="ExternalInput").ap()
        dum_out = nc.dram_tensor("dum_out", (128, nsl + 4), F16,
                                 kind="ExternalOutput").ap()

    with tile.TileContext(nc) as tc:
        with tc.tile_pool(name="res", bufs=1) as rpool, \
             tc.tile_pool(name="xf", bufs=xbufs) as xpool, \
             tc.tile_pool(name="xh", bufs=hbufs) as hpool, \
             tc.tile_pool(name="t16", bufs=tbufs) as tpool, \
             tc.tile_pool(name="sl", bufs=2) as slpool, \
             tc.tile_pool(name="psum", bufs=2, space="PSUM") as ppool, \
             tc.tile_pool(name="ionce", bufs=1) as ipool, \
             tc.tile_pool(name="small", bufs=2) as spool:

            wt_sb = rpool.tile([128, C * S], F16)
            bias_sb = rpool.tile([128, C], F32)
            iota16 = rpool.tile([128, S], F16)   # 0..2047 per partition

            ii = ipool.tile([128, S], I16, tag="iota_i")
            nc.gpsimd.iota(ii[:], pattern=[[1, S]], base=0,
                           channel_multiplier=0)
            nc.vector.tensor_copy(iota16[:], ii[:])
            # weight/bias loads ride the ACT HWDGE queue so the first slice's
            # input stream (SP queue) interleaves with them on the DMA device
            for c in range(C):
                nc.scalar.dma_start(wt_sb[:, c * S:(c + 1) * S],
                                    wt[c * 128:(c + 1) * 128, :])
            nc.scalar.dma_start(bias_sb[:],
                                bias.rearrange("(c p) -> p c", p=128))

            for it in range(nsl * repeat):
                n = it % nsl
                xn = xs[n]
                on = outs[n]

                m8 = slpool.tile([128, C * 8], F32, tag="m8")
                i8 = slpool.tile([128, C * 8], U32, tag="i8")
                gh = spool.tile([128, C], F16, tag="gh")
                psum_t = ppool.tile([128, C * C], F32, tag="psum_y")
                x16blks = []
                for c in range(C):
                    xt = xpool.tile([128, S], F32, tag="x")
                    nc.sync.dma_start(xt[:], xn[c * 128:(c + 1) * 128, :])
                    # row max then exact first-occurrence index (both DVE:
                    # the only engine the real ISA gives for scans)
                    nc.vector.max(m8[:, c * 8:(c + 1) * 8], xt[:])
                    nc.vector.max_index(i8[:, c * 8:(c + 1) * 8],
                                        m8[:, c * 8:(c + 1) * 8], xt[:])
                    if c % 2 == 0:
                        x16 = hpool.tile([128, 2 * S], F16, tag="x16")
                        x16blks.append(x16)
                    nc.scalar.activation(x16[:, (c % 2) * S:(c % 2 + 1) * S],
                                         xt[:], AF.Copy)
                    # feed PE as soon as this chunk's row max is known
                    # (fp16 cast of the max rides ACT, not the busy DVE)
                    nc.scalar.activation(gh[:, c:c + 1],
                                         m8[:, c * 8:c * 8 + 1], AF.Copy)
                    for tci in range(C):
                        nc.tensor.matmul(
                            psum_t[:, c * C + tci: c * C + tci + 1],
                            wt_sb[:, c * S + tci * 128:
                                  c * S + (tci + 1) * 128],
                            gh[:, c:c + 1],
                            start=True, stop=True)

                # col-0 strided views [128, C, 1] of the per-chunk top-8 tiles
                m3 = m8[:].rearrange("p (c e) -> p c e", e=8)[:, :, 0:1]
                i3 = i8[:].rearrange("p (c e) -> p c e", e=8)[:, :, 0:1]

                y_pre = spool.tile([128, C], F32, tag="ypre")
                nc.vector.reduce_sum(
                    y_pre[:].rearrange("p (t o) -> p t o", o=1),
                    psum_t[:].rearrange("p (s t) -> p t s", s=C),
                    axis=mybir.AxisListType.X)
                nc.vector.tensor_add(y_pre[:], y_pre[:], bias_sb[:])
                yv = spool.tile([128, C], F32, tag="yv")
                nc.scalar.activation(yv[:], y_pre[:], AF.Gelu)

                dltf = spool.tile([128, C], F32, tag="dlt")
                nc.vector.tensor_sub(
                    dltf[:].rearrange("p (c o) -> p c o", o=1),
                    yv[:].rearrange("p (c o) -> p c o", o=1), m3)
                idxf = spool.tile([128, C], F32, tag="idx")
                nc.vector.tensor_copy(
                    idxf[:].rearrange("p (c o) -> p c o", o=1), i3)

                for c in range(C):
                    t16 = tpool.tile([128, S], F16, tag="t16")
                    nc.vector.tensor_scalar(
                        t16[:], iota16[:], idxf[:, c:c + 1],
                        dltf[:, c:c + 1],
                        op0=ALU.is_equal, op1=ALU.mult)
                    blk = x16blks[c // 2]
                    half = (c % 2) * S
                    use_dma = (c >= C + dma_apply) if dma_apply < 0 \
                        else (c < dma_apply)
                    if use_dma:
                        # DVE is the bottleneck: ride the DMA slack instead —
                        # SWDGE CCE accumulate-add folds t16 into the block
                        nc.gpsimd.dma_start(blk[:, half:half + S], t16[:],
                                            accum_op=ALU.add)
                    else:
                        nc.vector.tensor_add(blk[:, half:half + S],
                                             blk[:, half:half + S], t16[:])
                    if c % 2 == 1:
                        b = c // 2
                        nc.scalar.dma_start(
                            on[b * 256:(b + 1) * 256, :].rearrange(
                                "(a p) m -> p a m", p=128),
                            blk[:].rearrange("p (a m) -> p a m", a=2))

            if internal_io:
                live = spool.tile([128, nsl + 4], F16, tag="live")
                nc.gpsimd.dma_start(live[:, nsl:], dum_in[:])
                for n in range(nsl):
                    nc.sync.dma_start(live[:, n:n + 1], outs[n][:128, 0:1])
                nc.sync.dma_start(dum_out[:], live[:])

    nc.compile()
    return nc


_NC_CACHE = {}


def _get_nc():
    if "nc" not in _NC_CACHE:
        _NC_CACHE["nc"] = _build()
    return _NC_CACHE["nc"]


def _make_in_maps(x, W, b):
    x = np.ascontiguousarray(np.asarray(x, dtype=np.float32))
    W = np.asarray(W, dtype=np.float32)
    b = np.ascontiguousarray(np.asarray(b, dtype=np.float32))
    wt = np.ascontiguousarray(W.T.astype(np.float16))

    xf = x.reshape(-1, S, S)
    assert xf.shape[0] == N_CORES * NSL
    in_maps = []
    for core in range(N_CORES):
        in_maps.append({
            "xs": xf[core * NSL:(core + 1) * NSL],
            "wt": wt,
            "bias": b,
        })
    return in_maps


def _run(in_maps, **kwargs):
    nc = _get_nc()
    return bass_utils.run_bass_kernel_spmd(
        nc, in_maps, core_ids=list(range(N_CORES)), **kwargs)


def kernel(x, W, b):
    shape = np.asarray(x).shape
    res = _run(_make_in_maps(x, W, b))
    parts = [res.results[core][f"out{n}"]
             for core in range(N_CORES) for n in range(NSL)]
    return np.stack(parts).astype(np.float32).reshape(shape)


# revision 8
# speedup vs baseline: 1.1182x; 1.0598x over previous
"""Trainium2 kernel for nn_PatternsOfThinkingBlock (topk_masking), v2.

reference:
  idx = argmax(x, -1); gathered = x[..., idx]   (gathered == row max)
  y = gelu(einsum('bhs,ts->bht', gathered, W) + b)   (exact erf gelu)
  out = x with x[b,h,s,idx[b,h,s]] = y[b,h,s]

Strategy vs the v1 baseline: the output tolerance (2e-2 of absmax) lets the
output travel as fp16 (roundoff ~4e-3 absolute), halving write traffic; the
max/argmax stay exact in f32.  Pure data parallel over the 32 (b,h) slices,
4 per core.  Per [2048, 2048] slice, per 128-row chunk of the stream:
  - HWDGE DMA-in (f32, 1 MB per chunk)
  - DVE max8 -> exact f32 row max; DVE max_index -> exact FIRST-occurrence
    argmax (the dataset has 4 rows with exactly tied f32 maxima; first
    occurrence must match the reference argmax)
  - ACT copies the chunk to fp16 (casting off the busy DVE)
Slice level: PE matvec z = W @ rowmax against resident fp16 W.T chunks
(fed per chunk as each max lands), gelu+bias on ACT/DVE, then per chunk a
4x-mode tensor_scalar builds onehot(idx)*(y - m) in fp16 from a resident
iota and a 2x tensor_tensor adds it into the fp16 chunk in place; HWDGE
DMA-out (fp16, 1 MB per 2-chunk block).  DMA moves 16.8 MB in + 8.4 MB out
per slice (~71 us serial), under the DVE scan floor of ~98 us/slice --
the two first-occurrence-exact scans are the binding constraint on TRN2's
real ISA (no elementwise ops on Pool, no fused max+index op).
"""

import numpy as np

import concourse.bacc as bacc
import concourse.mybir as mybir
import concourse.tile as tile
from concourse import bass_utils

F32 = mybir.dt.float32
F16 = mybir.dt.float16
I16 = mybir.dt.int16
U32 = mybir.dt.uint32

AF = mybir.ActivationFunctionType
ALU = mybir.AluOpType

S = 2048
NSL = 4            # bh slices per core
N_CORES = 8
C = S // 128       # 16 row chunks per slice


def _build(n_cores=N_CORES, repeat=1, internal_io=False, nsl=NSL,
           xbufs=6, hbufs=13, nd=5, tbufs=6):
    nc = bacc.Bacc("TRN2", target_bir_lowering=False, debug=False,
                   num_devices=n_cores)

    big_in = "Internal" if internal_io else "ExternalInput"
    big_out = "Internal" if internal_io else "ExternalOutput"
    xs = nc.dram_tensor("xs", (nsl, S, S), F32, kind=big_in).ap()
    wt = nc.dram_tensor("wt", (S, S), F16, kind=big_in).ap()
    bias = nc.dram_tensor("bias", (S,), F32, kind=big_in).ap()
    outs = [nc.dram_tensor(f"out{n}", (S, S), F16, kind=big_out).ap()
            for n in range(nsl)]
    if internal_io:
        dum_in = nc.dram_tensor("dum_in", (128, 4), F32,
                                kind="ExternalInput").ap()
        dum_out = nc.dram_tensor("dum_out", (128, nsl + 4), F16,
                                 kind="ExternalOutput").ap()

    with tile.TileContext(nc) as tc:
        with tc.tile_pool(name="res", bufs=1) as rpool, \
             tc.tile_pool(name="xf", bufs=xbufs) as xpool, \
             tc.tile_pool(name="xh", bufs=hbufs) as hpool, \
             tc.tile_pool(name="t16", bufs=tbufs) as tpool, \
             tc.tile_pool(name="sl", bufs=2) as slpool, \
             tc.tile_pool(name="psum", bufs=2, space="PSUM") as ppool, \
             tc.tile_pool(name="ionce", bufs=1) as ipool, \
             tc.tile_pool(name="small", bufs=2) as spool:

            wt_sb = rpool.tile([128, C * S], F16)
            bias_sb = rpool.tile([128, C], F32)
            iota16 = rpool.tile([128, S], F16)   # 0..2047 per partition

            ii = ipool.tile([128, S], I16, tag="iota_i")
            nc.gpsimd.iota(ii[:], pattern=[[1, S]], base=0,
                           channel_multiplier=0)
            nc.vector.tensor_copy(iota16[:], ii[:])
            # weight/bias loads ride the ACT HWDGE queue so the first slice's
            # input stream (SP queue) interleaves with them on the DMA device
            for c in range(C):
                nc.scalar.dma_start(wt_sb[:, c * S:(c + 1) * S],
                                    wt[c * 128:(c + 1) * 128, :])
            nc.scalar.dma_start(bias_sb[:],
                                bias.rearrange("(c p) -> p c", p=128))

            def tail_sweep2(st):
                on, m8, i8, psum_t, x16add = st
                m3 = m8[:].rearrange("p (c e) -> p c e", e=8)[:, :, 0:1]
                i3 = i8[:].rearrange("p (c e) -> p c e", e=8)[:, :, 0:1]
                y_pre = spool.tile([128, C], F32, tag="ypre")
                nc.vector.reduce_sum(
                    y_pre[:].rearrange("p (t o) -> p t o", o=1),
                    psum_t[:].rearrange("p (s t) -> p t s", s=C),
                    axis=mybir.AxisListType.X)
                nc.vector.tensor_add(y_pre[:], y_pre[:], bias_sb[:])
                yv = spool.tile([128, C], F32, tag="yv")
                nc.scalar.activation(yv[:], y_pre[:], AF.Gelu)
                dltf = spool.tile([128, C], F32, tag="dlt")
                nc.vector.tensor_sub(
                    dltf[:].rearrange("p (c o) -> p c o", o=1),
                    yv[:].rearrange("p (c o) -> p c o", o=1), m3)
                idxf = spool.tile([128, C], F32, tag="idx")
                nc.vector.tensor_copy(
                    idxf[:].rearrange("p (c o) -> p c o", o=1), i3)
                # CCE chunks first: their accumulates spread across the
                # DMA timeline instead of clumping after the DVE adds
                for c in list(range(nd, C)) + list(range(nd)):
                    t16 = tpool.tile([128, S], F16, tag="t16")
                    nc.vector.tensor_scalar(
                        t16[:], iota16[:], idxf[:, c:c + 1],
                        dltf[:, c:c + 1],
                        op0=ALU.is_equal, op1=ALU.mult)
                    if c >= nd:
                        nc.gpsimd.dma_start(on[c * 128:(c + 1) * 128, :],
                                            t16[:], accum_op=ALU.add)
                    else:
                        x16 = x16add[c]
                        nc.vector.tensor_add(x16[:], x16[:], t16[:])
                        nc.scalar.dma_start(on[c * 128:(c + 1) * 128, :],
                                            x16[:])

            pending = None
            for it in range(nsl * repeat):
                n = it % nsl
                xn = xs[n]
                on = outs[n]

                m8 = slpool.tile([128, C * 8], F32, tag="m8")
                i8 = slpool.tile([128, C * 8], U32, tag="i8")
                gh = spool.tile([128, C], F16, tag="gh")
                psum_t = ppool.tile([128, C * C], F32, tag="psum_y")
                x16blks = []
                for c in range(C):
                    xt = xpool.tile([128, S], F32, tag="x")
                    nc.sync.dma_start(xt[:], xn[c * 128:(c + 1) * 128, :])
                    # row max then exact first-occurrence index (both DVE:
                    # the only engine the real ISA gives for scans)
                    nc.vector.max(m8[:, c * 8:(c + 1) * 8], xt[:])
                    nc.vector.max_index(i8[:, c * 8:(c + 1) * 8],
                                        m8[:, c * 8:(c + 1) * 8], xt[:])
                    x16 = hpool.tile([128, S], F16, tag="x16")
                    x16blks.append(x16)
                    nc.scalar.activation(x16[:], xt[:], AF.Copy)
                    if c >= nd:
                        # CCE-routed chunk: base write goes out NOW; the
                        # onehot lands later via a DRAM CCE accumulate (Tile
                        # orders the RMW after this write). Frees the tile.
                        nc.scalar.dma_start(on[c * 128:(c + 1) * 128, :],
                                            x16[:])
                    # feed PE as soon as this chunk's row max is known
                    # (fp16 cast of the max rides ACT, not the busy DVE)
                    nc.scalar.activation(gh[:, c:c + 1],
                                         m8[:, c * 8:c * 8 + 1], AF.Copy)
                    for tci in range(C):
                        nc.tensor.matmul(
                            psum_t[:, c * C + tci: c * C + tci + 1],
                            wt_sb[:, c * S + tci * 128:
                                  c * S + (tci + 1) * 128],
                            gh[:, c:c + 1],
                            start=True, stop=True)

                # software pipeline: the previous slice's matvec tail and
                # apply sweep are emitted AFTER this slice's scan ops, so the
                # DVE never stalls on the gelu chain at slice boundaries
                st = (on, m8, i8, psum_t, x16blks[:nd])
                if pending is not None:
                    tail_sweep2(pending)
                pending = st

            if pending is not None:
                tail_sweep2(pending)

            if internal_io:
                live = spool.tile([128, nsl + 4], F16, tag="live")
                nc.gpsimd.dma_start(live[:, nsl:], dum_in[:])
                for n in range(nsl):
                    nc.sync.dma_start(live[:, n:n + 1], outs[n][:128, 0:1])
                nc.sync.dma_start(dum_out[:], live[:])

    nc.compile()
    return nc


_NC_CACHE = {}


def _get_nc():
    if "nc" not in _NC_CACHE:
        _NC_CACHE["nc"] = _build()
    return _NC_CACHE["nc"]


def _make_in_maps(x, W, b):
    x = np.ascontiguousarray(np.asarray(x, dtype=np.float32))
    W = np.asarray(W, dtype=np.float32)
    b = np.ascontiguousarray(np.asarray(b, dtype=np.float32))
    wt = np.ascontiguousarray(W.T.astype(np.float16))

    xf = x.reshape(-1, S, S)
    assert xf.shape[0] == N_CORES * NSL
    in_maps = []
    for core in range(N_CORES):
        in_maps.append({
            "xs": xf[core * NSL:(core + 1) * NSL],
            "wt": wt,
            "bias": b,
        })
    return in_maps


def _run(in_maps, **kwargs):
    nc = _get_nc()
    return bass_utils.run_bass_kernel_spmd(
        nc, in_maps, core_ids=list(range(N_CORES)), **kwargs)


def kernel(x, W, b):
    shape = np.asarray(x).shape
    res = _run(_make_in_maps(x, W, b))
    parts = [res.results[core][f"out{n}"]
             for core in range(N_CORES) for n in range(NSL)]
    return np.stack(parts).astype(np.float32).reshape(shape)
